# revision 25
# baseline (speedup 1.0000x reference)
"""Trainium2 Bass kernel for nn_Encoder_60112362275055 (GRU with skip connections).

B=64, T=512, X=256, H=1024, skip_size=5. Output = 2 * h_{T-1}  -> [64, 1024].

Data-parallel over batch (8 cores x B_local=8), zero cross-core traffic.
The skip structure (w1[t]==0 drops the h_{t-1} dependency) turns the scan
into a DAG; only the 318 ancestors of t=T-1 are computed (87 levels,
max width 8 -> M <= 64 rows per level per core). ~880 us HW exec
(vs 2225 us for the v2 half-split baseline).

Final design:
- partition-packed layout: h-cols 0:512 of every per-node vector live on
  psum/sbuf partitions [0:M], h-cols 512:1024 on [64:64+M]; every
  elementwise tail op covers the whole H in one [128, *] instruction.
- per level, bank-major gate streams (r-h1/n-h1/r-h2/n-h2 chunk halves,
  then h_blend identity matmuls, then z) so the r/n tail chain overlaps
  the z streams and the gpsimd blend of the next level's stationary gets
  ~1.7us of lead time.
- xi_z folded into the z psum via [I64;0]/[0;I64] selector matmuls
  (keeps the end-of-level chain to z-sig -> mul -> add).
- hist^T (stationary source) built by regular identity matmuls, NOT
  transpose-mode: consecutive transpose-mode/regular MMs whose stationary
  operands sit at different partition offsets wedge the PE on this HW, so
  every stationary is K=128 at base partition 0.
- phase 1 (xi = x @ W_ih.T + bias) is emitted LAZILY: (M-tile, bank) jobs
  are interleaved into the per-level tail bubbles (matmuls mid-tail after
  dependency-timed triggers, adds/DMA at level end), which both hides the
  ~150us prologue and keeps the PE activity monitor from re-throttling
  the clock (cold 1.2 GHz matmuls measured at every level start).
- tiny "warm" matmuls whose moving operands are tail outputs fire
  mid-bubble for the same reason.
- all matmul operands and the tail run in bf16 (psum f32); rel err 0.0097
  vs the f32 reference (gate 2e-2).
"""

import os
import sys

import numpy as np

sys.path.insert(0, "/opt/trn_rl_repo")

NLEV_CAP = int(os.environ.get("NLEV_CAP", "0"))  # 0 = all levels

import ml_dtypes

import concourse.bacc as bacc
import concourse.mybir as mybir
from concourse import tile
from concourse.bass_utils import run_bass_kernel_spmd

B, T, X, H = 64, 512, 256, 1024
SKIP = 5
NCORES = 8
BL = B // NCORES  # 8
G3 = 3 * H  # 3072
KC = H // 128  # 8 K-chunks
NLEVH = 16  # rolling history depth in levels (max dep distance ~10)
WCOL = 64  # node-cols per (level, chunk) hist region (max M=64)
HALF = H // 2  # 512

f32 = mybir.dt.float32
bf16 = mybir.dt.bfloat16
AF = mybir.ActivationFunctionType
bfnp = ml_dtypes.bfloat16


def _skip_plan(T, skip_size):
    slots = np.zeros(T, np.int32)
    use_zero = np.zeros(T, np.float32)
    for i in range(T):
        if i < skip_size:
            if 2 * i < skip_size:
                use_zero[i] = 1.0
            else:
                slots[i] = (skip_size - i) - 1
        else:
            if i - skip_size < skip_size:
                use_zero[i] = 1.0
            else:
                slots[i] = 2 * skip_size - 1
    return slots, use_zero


def _plan(w1, w2):
    """Topological levels over the ancestor set of t=T-1."""
    slots, use_zero = _skip_plan(T, SKIP)
    d1 = np.full(T, -1, np.int64)
    d2 = np.full(T, -1, np.int64)
    for t in range(T):
        if w1[t] == 1 and t - 1 >= 0:
            d1[t] = t - 1
        if w2[t] == 1 and use_zero[t] == 0.0 and t - 1 - slots[t] >= 0:
            d2[t] = t - 1 - slots[t]
    anc = set()
    stack = [T - 1]
    while stack:
        t = stack.pop()
        if t in anc:
            continue
        anc.add(t)
        for d in (d1[t], d2[t]):
            if d >= 0 and d not in anc:
                stack.append(int(d))
    lv_of = {}
    for t in sorted(anc):
        deps = [d for d in (d1[t], d2[t]) if d >= 0]
        lv_of[t] = 1 + max((lv_of[d] for d in deps), default=-1)
    nlev = max(lv_of.values()) + 1
    levels = [[] for _ in range(nlev)]
    idx_of = {}
    for t in sorted(anc):
        idx_of[t] = len(levels[lv_of[t]])
        levels[lv_of[t]].append(t)
    order = [t for lv in levels for t in lv]
    maxdist = 0
    for t in sorted(anc):
        for d in (d1[t], d2[t]):
            if d >= 0:
                maxdist = max(maxdist, int(lv_of[t] - lv_of[d]))
    assert maxdist + 3 <= NLEVH, (maxdist, "NLEVH too small")
    assert max(len(lv) for lv in levels) * BL <= 64
    assert lv_of[T - 1] == nlev - 1
    return levels, order, lv_of, idx_of, d1, d2


def _build(levels, lv_of, idx_of, d1, d2, order):
    nc = bacc.Bacc(None)

    NROW = len(order)
    MT = (NROW * BL + 127) // 128  # phase-1 M-tiles
    NPAD = MT * 128

    xs_d = nc.dram_tensor("xs", [2, 128, NPAD], bf16, kind="ExternalInput")
    wih_d = nc.dram_tensor("wih", [2, 128, G3], bf16, kind="ExternalInput")
    whh_d = nc.dram_tensor("whh", [KC, 128, G3], bf16, kind="ExternalInput")
    biasg_d = nc.dram_tensor("biasg", [128, G3], f32, kind="ExternalInput")
    bias2_d = nc.dram_tensor("bias2", [128, HALF], bf16, kind="ExternalInput")
    # ident cols 0:128 = I_128; 128:192 = [I64;0]; 192:256 = [0;I64]
    ident_d = nc.dram_tensor("ident", [128, 256], bf16, kind="ExternalInput")
    out_d = nc.dram_tensor("out", [BL, H], f32, kind="ExternalOutput")
    xi_d = nc.dram_tensor("xi_scratch", [NPAD, G3], bf16)

    NLEV = len(levels)
    if NLEV_CAP:
        NLEV = min(NLEV, NLEV_CAP)
        levels = levels[:NLEV]
    t_out = T - 1

    with tile.TileContext(nc) as tc:
        with (
            tc.tile_pool(name="rec", bufs=1) as rpool,
            tc.tile_pool(name="ps", bufs=7, space="PSUM") as pspool,
            tc.tile_pool(name="junk", bufs=1, space="PSUM") as junkpool,
        ):
            whh = rpool.tile([128, KC * G3], bf16)  # 48 KB/partition
            b2bc = rpool.tile([128, HALF], bf16)
            nc.sync.dma_start(b2bc[:], bias2_d[:])
            ident = rpool.tile([128, 256], bf16)
            nc.sync.dma_start(ident[:], ident_d[:])
            # hist^T: [128, slot(16) x chunk(8) x node-col(64)] bf16
            hist = rpool.tile([128, NLEVH * KC * WCOL], bf16)
            hv = hist[:].rearrange("p (s c j) -> p s c j", s=NLEVH, c=KC)

            # ---------- phase 1 (lazy): xi = x @ W_ih.T + bias ----------
            # emitted as (M-tile, bank) jobs interleaved into the level-loop
            # tail bubbles; they fill PE idle time and keep the clock warm
            wih = rpool.tile([128, 2 * G3], bf16)
            nc.sync.dma_start(
                wih[:].rearrange("p (k f) -> p k f", k=2),
                wih_d.rearrange("k p f -> p k f"),
            )
            xs = rpool.tile([128, 2 * NPAD], bf16)
            nc.sync.dma_start(
                xs[:].rearrange("p (k f) -> p k f", k=2),
                xs_d.rearrange("k p f -> p k f"),
            )
            biasg = rpool.tile([128, G3], f32)
            nc.sync.dma_start(biasg[:], biasg_d[:])
            # whh load on the SWDGE queue so the big transfer streams in
            # parallel with the sync-queue phase-1 flow (xi writes/loads)
            nc.gpsimd.dma_start(
                whh[:].rearrange("p (k f) -> p k f", k=KC),
                whh_d.rearrange("k p f -> p k f"),
            )
            p1sb_rot = [rpool.tile([128, 512], bf16, name=f"p1sb{i}") for i in range(3)]
            p1_state = [0]

            p1_pending = []

            def emit_p1_mm():
                j = p1_state[0]
                if j >= MT * 6 or len(p1_pending) >= 2:
                    return False
                p1_state[0] = j + 1
                m, nb = divmod(j, 6)
                ps = pspool.tile([128, 512], f32, tag="ps", name="p1ps")
                for k in range(2):
                    nc.tensor.matmul(
                        ps[:],
                        xs[:, k * NPAD + m * 128 : k * NPAD + (m + 1) * 128],
                        wih[:, k * G3 + nb * 512 : k * G3 + (nb + 1) * 512],
                        start=(k == 0),
                        stop=(k == 1),
                    )
                p1_pending.append((j, m, nb, ps))
                return True

            def flush_p1():
                while p1_pending:
                    j, m, nb, ps = p1_pending.pop(0)
                    sb = p1sb_rot[j % 3]
                    nc.vector.tensor_add(
                        sb[:], ps[:], biasg[:, nb * 512 : (nb + 1) * 512]
                    )
                    nc.sync.dma_start(
                        xi_d[m * 128 : (m + 1) * 128, nb * 512 : (nb + 1) * 512],
                        sb[:],
                    )

            def emit_p1_job():
                if emit_p1_mm():
                    flush_p1()
                    return True
                return False

            def p1_rows_done():
                return (p1_state[0] // 6) * 128

            # ---------- phase 2: recurrence over DAG levels ----------
            with (
                tc.tile_pool(name="rzp", bufs=2) as rzp,
                tc.tile_pool(name="npp", bufs=2) as npp,
                tc.tile_pool(name="hbm", bufs=2) as hbmp,
                tc.tile_pool(name="hnp", bufs=3) as hnp,
                tc.tile_pool(name="outp", bufs=1) as opool,
            ):
                hbl_rot = [
                    rpool.tile([128, KC * WCOL], bf16, name=f"hblr{i}")
                    for i in range(3)
                ]
                xi_rot = [
                    rpool.tile([128, 3 * HALF], bf16, name=f"xir{i}")
                    for i in range(3)
                ]

                def hist_pair(t, c0):
                    """[128, 2, BL] view of dep t's h^T chunks c0, c0+1."""
                    sl = int(lv_of[t]) % NLEVH
                    j = idx_of[t]
                    return hv[:, sl, c0 : c0 + 2, j * BL : (j + 1) * BL]

                def emit_blend(eng, hbl, nodes, half):
                    """Blend deps into hbl chunks {0,1,4,5} (half 0) or
                    {2,3,6,7} (half 1); two 2-chunk ops per node."""
                    hblv = hbl[:].rearrange("p (c j) -> p c j", c=KC)
                    for c0 in ((0, 4) if half == 0 else (2, 6)):
                        for i, t in enumerate(nodes):
                            dst = hblv[:, c0 : c0 + 2, i * BL : (i + 1) * BL]
                            a, b2 = int(d1[t]), int(d2[t])
                            if a < 0 and b2 < 0:
                                eng.memset(dst, 0.0)
                            elif a >= 0 and b2 >= 0:
                                eng.tensor_add(
                                    dst, hist_pair(a, c0), hist_pair(b2, c0)
                                )
                            else:
                                eng.tensor_copy(
                                    dst, hist_pair(a if a >= 0 else b2, c0)
                                )

                # row offset of each level's xi rows (level-sorted, tight)
                srow = [0]
                for lvw in levels:
                    srow.append(srow[-1] + len(lvw))

                junk = junkpool.tile([128, 512], f32, tag="junk")

                def warm(mv):
                    # tiny matmul whose moving operand is a tail output: it
                    # fires mid-bubble (right after that output lands) and
                    # keeps the PE activity monitor from re-throttling
                    nc.tensor.matmul(
                        junk[0:64, 0:64], whh[:, 0:64], mv[:, 0:64],
                        start=True, stop=True, skip_group_check=True,
                    )

                # pre-loop: zero all rotation buffers (garbage lanes must
                # hold written data for the full-64-wide matmul reads)
                for tbuf in hbl_rot:
                    nc.vector.memset(tbuf[:, :], 0.0)
                for tbuf in xi_rot:
                    nc.gpsimd.memset(tbuf[:, :], 0.0)
                hbl_cur = hbl_rot[0]
                emit_blend(nc.vector, hbl_cur, levels[0], 0)
                emit_blend(nc.gpsimd, hbl_cur, levels[0], 1)

                # 3 xi tiles up front (covers first ~5 levels); after the
                # memsets/blend-0 so they don't block level-0 on the DVE queue
                for _ in range(18):
                    emit_p1_job()

                # chunk stream order: h1-chunks first (their blend lands first)
                CORD = (0, 1, 4, 5, 2, 3, 6, 7)

                for lv, nodes in enumerate(levels):
                    w = len(nodes)
                    M = BL * w
                    sl = lv % NLEVH
                    hbl = hbl_cur

                    # xi rows for this level (SWDGE, off the HWDGE queues)
                    # [0:M] rows = gate cols 0:1536 (rA zA nA),
                    # [64:64+M]  = gate cols 1536:3072 (rB zB nB)
                    xi = xi_rot[lv % 3]
                    r0 = srow[lv] * BL
                    nc.sync.dma_start(xi[0:M, :], xi_d[r0 : r0 + M, 0 : 3 * HALF])
                    nc.sync.dma_start(
                        xi[64 : 64 + M, :], xi_d[r0 : r0 + M, 3 * HALF : G3]
                    )

                    ps_hb = pspool.tile([128, 512], f32, tag="ps", name="ps_hb")
                    ps_r = pspool.tile([128, 512], f32, tag="ps", name="ps_r")
                    ps_n = pspool.tile([128, 512], f32, tag="ps", name="ps_n")
                    ps_z = pspool.tile([128, 512], f32, tag="ps", name="ps_z")

                    def gate_stream(psb, gcol, lo=0, hi=512):
                        for ci, c in enumerate(CORD):
                            lhsT = hbl[:, c * WCOL : (c + 1) * WCOL]
                            st = ci == 0
                            sp = ci == KC - 1
                            nc.tensor.matmul(
                                psb[0:64, lo:hi],
                                lhsT,
                                whh[:, c * G3 + gcol * 512 + lo : c * G3 + gcol * 512 + hi],
                                start=st,
                                stop=sp,
                                skip_group_check=True,
                            )
                            nc.tensor.matmul(
                                psb[64:128, lo:hi],
                                lhsT,
                                whh[:, c * G3 + 1536 + gcol * 512 + lo : c * G3 + 1536 + gcol * 512 + hi],
                                start=st,
                                stop=sp,
                                skip_group_check=True,
                            )

                    # ---- r/n gates: h1 chunks of both banks first, so the
                    # gpsimd blend-h2 of this level gets ~1.7us of lead ----
                    def gate_stream_half(psb, gcol, half):
                        for ci, c in enumerate(CORD):
                            if (ci < 4) != (half == 0):
                                continue
                            lhsT = hbl[:, c * WCOL : (c + 1) * WCOL]
                            st = ci == 0
                            sp = ci == KC - 1
                            nc.tensor.matmul(
                                psb[0:64, :],
                                lhsT,
                                whh[:, c * G3 + gcol * 512 : c * G3 + (gcol + 1) * 512],
                                start=st, stop=sp, skip_group_check=True,
                            )
                            nc.tensor.matmul(
                                psb[64:128, :],
                                lhsT,
                                whh[:, c * G3 + 1536 + gcol * 512 : c * G3 + 1536 + (gcol + 1) * 512],
                                start=st, stop=sp, skip_group_check=True,
                            )

                    gate_stream_half(ps_r, 0, 0)
                    gate_stream_half(ps_n, 2, 0)
                    gate_stream_half(ps_r, 0, 1)
                    gate_stream_half(ps_n, 2, 1)

                    # ---- h_blend batch layout: PE transposes of hbl ----
                    # chunk c (A: 0-3 / B: 4-7) -> ps_hb rows [0:M]/[64:64+M]
                    for c in range(4):
                        nc.tensor.matmul(
                            ps_hb[0:64, c * 128 : (c + 1) * 128],
                            hbl[:, c * WCOL : (c + 1) * WCOL],
                            ident[:, 0:128],
                            start=True, stop=True, skip_group_check=True,
                        )
                        nc.tensor.matmul(
                            ps_hb[64:128, c * 128 : (c + 1) * 128],
                            hbl[:, (c + 4) * WCOL : (c + 5) * WCOL],
                            ident[:, 0:128],
                            start=True, stop=True, skip_group_check=True,
                        )

                    rz = rzp.tile([128, 2 * 512], bf16, tag="rz")
                    np_ = npp.tile([128, 512], bf16, tag="np")
                    hbmn = hbmp.tile([128, 512], bf16, tag="hbmn")
                    hnew = hnp.tile([128, 512], bf16, tag="hnew")

                    # r = sigmoid(ps_r + xi_r)  (overlaps z streams)
                    nc.vector.tensor_add(rz[:, 0:512], ps_r[:, :], xi[:, 0:512])
                    nc.scalar.activation(rz[:, 0:512], rz[:, 0:512], AF.Sigmoid)

                    # ---- z gate, xi_z folded via identity selectors ----
                    gate_stream(ps_z, 1)
                    nc.tensor.matmul(
                        ps_z[0:64, :],
                        ident[:, 128:192],
                        xi[:, 512:1024],
                        start=False, stop=False, skip_group_check=True,
                    )
                    nc.tensor.matmul(
                        ps_z[64:128, :],
                        ident[:, 192:256],
                        xi[:, 512:1024],
                        start=False, stop=False, skip_group_check=True,
                    )

                    # hb psum -> sbuf early (off the critical chain, on ACT)
                    nc.scalar.copy(hbmn[:, :], ps_hb[:, :])
                    # n = tanh(xi_n + r * (ps_n + b_hh_n))
                    nc.vector.tensor_add(np_[:, :], ps_n[:, :], b2bc[:, :])
                    nc.vector.tensor_mul(np_[:, :], np_[:, :], rz[:, 0:512])
                    nc.vector.tensor_add(np_[:, :], np_[:, :], xi[:, 1024:1536])
                    nc.scalar.activation(np_[:, 0:256], np_[:, 0:256], AF.Tanh)
                    nc.scalar.activation(np_[:, 256:512], np_[:, 256:512], AF.Tanh)
                    nc.scalar.activation(rz[:, 512:768], ps_z[:, 0:256], AF.Sigmoid)
                    nc.scalar.activation(rz[:, 768:1024], ps_z[:, 256:512], AF.Sigmoid)
                    warm(np_[:, 0:256])
                    if not emit_p1_mm():
                        nc.tensor.matmul(
                            junk[0:64, 0:128], whh[:, 0:64], whh[:, 0:128],
                            start=True, stop=True, skip_group_check=True,
                        )
                    warm(rz[:, 512:768])
                    if not emit_p1_mm():
                        nc.tensor.matmul(
                            junk[0:64, 0:128], whh[:, 0:64], whh[:, 0:128],
                            start=True, stop=True, skip_group_check=True,
                        )

                    # h_new = n + z*(hb-n), in halves; each half's transposes,
                    # hist copies and next-level blends follow immediately
                    for hh in range(2):
                        cs = slice(hh * 256, (hh + 1) * 256)
                        nc.vector.tensor_sub(hbmn[:, cs], hbmn[:, cs], np_[:, cs])
                        nc.vector.tensor_mul(
                            hbmn[:, cs], hbmn[:, cs],
                            rz[:, 512 + hh * 256 : 512 + (hh + 1) * 256],
                        )
                        nc.vector.tensor_add(hnew[:, cs], hbmn[:, cs], np_[:, cs])
                        warm(hnew[:, cs])

                        if lv + 1 < NLEV:
                            pt = pspool.tile([128, 256], f32, tag="ps", name="pt")
                            for k in range(2):
                                col = hh * 256 + k * 128
                                nc.tensor.matmul(
                                    pt[:, k * WCOL : (k + 1) * WCOL],
                                    hnew[:, col : col + 128],
                                    ident[:, 128:192],
                                    start=True, stop=True,
                                    skip_group_check=True,
                                )
                                nc.tensor.matmul(
                                    pt[:, (2 + k) * WCOL : (3 + k) * WCOL],
                                    hnew[:, col : col + 128],
                                    ident[:, 192:256],
                                    start=True, stop=True,
                                    skip_group_check=True,
                                )
                            # psum -> hist^T: chunks {hh*2, hh*2+1} and
                            # {hh*2+4, hh*2+5} -- one strided copy
                            base = sl * KC * WCOL
                            hd = hist[
                                :, base + hh * 128 : base + hh * 128 + 384
                            ].rearrange("p (g f) -> p g f", g=3)[:, 0:3:2, :]
                            nc.scalar.copy(
                                hd,
                                pt[:].rearrange("p (g f) -> p g f", g=2),
                            )
                            if hh == 0:
                                hbl_cur = hbl_rot[(lv + 1) % 3]

                    if lv + 1 < NLEV:
                        emit_blend(nc.vector, hbl_cur, levels[lv + 1], 0)
                        emit_blend(nc.gpsimd, hbl_cur, levels[lv + 1], 1)

                    flush_p1()
                    need = srow[min(lv + 5, NLEV)] * BL
                    emitted = 0
                    while p1_rows_done() < min(need + 128, NPAD) and emitted < 6:
                        if not emit_p1_job():
                            break
                        emitted += 1

                    if lv == NLEV - 1:
                        j = idx_of[t_out] if not NLEV_CAP else 0
                        outt = opool.tile([128, HALF], f32, tag="outt")
                        nc.vector.tensor_scalar_mul(outt[:, :], hnew[:, :], 2.0)
                        nc.sync.dma_start(
                            out_d[:, 0:HALF], outt[j * BL : (j + 1) * BL, :]
                        )
                        nc.sync.dma_start(
                            out_d[:, HALF:H], outt[64 + j * BL : 64 + (j + 1) * BL, :]
                        )

    nc.finalize()
    return nc


def kernel(**inputs):
    x = np.asarray(inputs["x"], np.float32)
    W_ih = np.asarray(inputs["W_ih"], np.float32)
    W_hh = np.asarray(inputs["W_hh"], np.float32)
    b_ih = np.asarray(inputs["b_ih"], np.float32)
    b_hh = np.asarray(inputs["b_hh"], np.float32)
    w1 = np.asarray(inputs["w1"], np.int32)
    w2 = np.asarray(inputs["w2"], np.int32)
    assert int(inputs["skip_size"]) == SKIP

    levels, order, lv_of, idx_of, d1, d2 = _plan(w1, w2)
    nc = _build(levels, lv_of, idx_of, d1, d2, order)

    NROW = len(order)
    MT = (NROW * BL + 127) // 128
    NPAD = MT * 128

    # gate-column permutation: [rA zA nA rB zB nB], A = cols 0:512 of a gate
    perm = np.concatenate(
        [
            np.arange(0, HALF),
            np.arange(H, H + HALF),
            np.arange(2 * H, 2 * H + HALF),
            np.arange(HALF, H),
            np.arange(H + HALF, 2 * H),
            np.arange(2 * H + HALF, 3 * H),
        ]
    )
    W_hh_p = W_hh[perm]
    W_ih_p = W_ih[perm]
    bias = (b_ih + b_hh).copy()
    bias[2 * H :] = b_ih[2 * H :]  # n-part: only b_ih (b_hh_n inside r*(.))
    bias = bias[perm]
    whh_t = np.ascontiguousarray(W_hh_p.T.reshape(KC, 128, G3)).astype(bfnp)
    wih_t = np.ascontiguousarray(W_ih_p.T.reshape(2, 128, G3)).astype(bfnp)
    biasg = np.broadcast_to(bias, (128, G3)).astype(np.float32).copy()
    # b_hh_n partition-packed: rows 0:64 = cols 0:512, rows 64:128 = 512:1024
    bias2g = np.zeros((128, HALF), dtype=bfnp)
    bias2g[0:64, :] = b_hh[2 * H : 2 * H + HALF].astype(bfnp)
    bias2g[64:128, :] = b_hh[2 * H + HALF : 3 * H].astype(bfnp)
    identh = np.zeros((128, 256), dtype=bfnp)
    identh[:, 0:128] = np.eye(128, dtype=bfnp)
    identh[0:64, 128:192] = np.eye(64, dtype=bfnp)
    identh[64:128, 192:256] = np.eye(64, dtype=bfnp)
    in_maps = []
    for c in range(NCORES):
        xc = x[c * BL : (c + 1) * BL]  # [8, T, X]
        xsrt = xc[:, order, :]  # pruned, level-sorted: [8, NROW, 256]
        xs = xsrt.transpose(2, 1, 0).reshape(2, 128, NROW * BL)
        xsp = np.zeros((2, 128, NPAD), np.float32)
        xsp[:, :, : NROW * BL] = xs
        in_maps.append(
            {
                "xs": xsp.astype(bfnp),
                "wih": wih_t,
                "whh": whh_t,
                "biasg": biasg,
                "bias2": bias2g,
                "ident": identh,
            }
        )
    res = None
    for attempt in range(3):
        try:
            res = run_bass_kernel_spmd(nc, in_maps, core_ids=list(range(NCORES)))
            break
        except Exception:
            if attempt == 2:
                raise
    if getattr(res, "exec_time_ns", None):
        print("HW exec time:", res.exec_time_ns, "ns")
    global LAST_RESULT
    LAST_RESULT = res
    out = np.concatenate([res.results[c]["out"] for c in range(NCORES)], axis=0)
    return np.asarray(out, np.float32)


LAST_RESULT = None


# revision 26
# speedup vs baseline: 1.0853x; 1.0853x over previous
"""Trainium2 Bass kernel for nn_Encoder_60112362275055 (GRU with skip connections).

B=64, T=512, X=256, H=1024, skip_size=5. Output = 2 * h_{T-1}  -> [64, 1024].

Data-parallel over batch (8 cores x B_local=8), zero cross-core traffic.
The skip structure (w1[t]==0 drops the h_{t-1} dependency) turns the scan
into a DAG; only the 318 ancestors of t=T-1 are computed (87 levels,
max width 8 -> M <= 64 rows per level per core). ~880 us HW exec
(vs 2225 us for the v2 half-split baseline).

Final design:
- partition-packed layout: h-cols 0:512 of every per-node vector live on
  psum/sbuf partitions [0:M], h-cols 512:1024 on [64:64+M]; every
  elementwise tail op covers the whole H in one [128, *] instruction.
- per level, bank-major gate streams (r-h1/n-h1/r-h2/n-h2 chunk halves,
  then h_blend identity matmuls, then z) so the r/n tail chain overlaps
  the z streams and the gpsimd blend of the next level's stationary gets
  ~1.7us of lead time.
- xi_z folded into the z psum via [I64;0]/[0;I64] selector matmuls
  (keeps the end-of-level chain to z-sig -> mul -> add).
- hist^T (stationary source) built by regular identity matmuls, NOT
  transpose-mode: consecutive transpose-mode/regular MMs whose stationary
  operands sit at different partition offsets wedge the PE on this HW, so
  every stationary is K=128 at base partition 0.
- phase 1 (xi = x @ W_ih.T + bias) is emitted LAZILY: (M-tile, bank) jobs
  are interleaved into the per-level tail bubbles (matmuls mid-tail after
  dependency-timed triggers, adds/DMA at level end), which both hides the
  ~150us prologue and keeps the PE activity monitor from re-throttling
  the clock (cold 1.2 GHz matmuls measured at every level start).
- tiny "warm" matmuls whose moving operands are tail outputs fire
  mid-bubble for the same reason.
- all matmul operands and the tail run in bf16 (psum f32); rel err 0.0097
  vs the f32 reference (gate 2e-2).
"""

import os
import sys

import numpy as np

sys.path.insert(0, "/opt/trn_rl_repo")

NLEV_CAP = int(os.environ.get("NLEV_CAP", "0"))  # 0 = all levels

import ml_dtypes

import concourse.bacc as bacc
import concourse.mybir as mybir
from concourse import tile
from concourse.bass_utils import run_bass_kernel_spmd

B, T, X, H = 64, 512, 256, 1024
SKIP = 5
NCORES = 8
BL = B // NCORES  # 8
G3 = 3 * H  # 3072
KC = H // 128  # 8 K-chunks
NLEVH = 16  # rolling history depth in levels (max dep distance ~10)
WCOL = 64  # node-cols per (level, chunk) hist region (max M=64)
HALF = H // 2  # 512

f32 = mybir.dt.float32
bf16 = mybir.dt.bfloat16
AF = mybir.ActivationFunctionType
bfnp = ml_dtypes.bfloat16


def _skip_plan(T, skip_size):
    slots = np.zeros(T, np.int32)
    use_zero = np.zeros(T, np.float32)
    for i in range(T):
        if i < skip_size:
            if 2 * i < skip_size:
                use_zero[i] = 1.0
            else:
                slots[i] = (skip_size - i) - 1
        else:
            if i - skip_size < skip_size:
                use_zero[i] = 1.0
            else:
                slots[i] = 2 * skip_size - 1
    return slots, use_zero


def _plan(w1, w2):
    """Topological levels over the ancestor set of t=T-1."""
    slots, use_zero = _skip_plan(T, SKIP)
    d1 = np.full(T, -1, np.int64)
    d2 = np.full(T, -1, np.int64)
    for t in range(T):
        if w1[t] == 1 and t - 1 >= 0:
            d1[t] = t - 1
        if w2[t] == 1 and use_zero[t] == 0.0 and t - 1 - slots[t] >= 0:
            d2[t] = t - 1 - slots[t]
    anc = set()
    stack = [T - 1]
    while stack:
        t = stack.pop()
        if t in anc:
            continue
        anc.add(t)
        for d in (d1[t], d2[t]):
            if d >= 0 and d not in anc:
                stack.append(int(d))
    lv_of = {}
    for t in sorted(anc):
        deps = [d for d in (d1[t], d2[t]) if d >= 0]
        lv_of[t] = 1 + max((lv_of[d] for d in deps), default=-1)
    nlev = max(lv_of.values()) + 1
    levels = [[] for _ in range(nlev)]
    idx_of = {}
    for t in sorted(anc):
        idx_of[t] = len(levels[lv_of[t]])
        levels[lv_of[t]].append(t)
    order = [t for lv in levels for t in lv]
    maxdist = 0
    for t in sorted(anc):
        for d in (d1[t], d2[t]):
            if d >= 0:
                maxdist = max(maxdist, int(lv_of[t] - lv_of[d]))
    assert maxdist + 3 <= NLEVH, (maxdist, "NLEVH too small")
    assert max(len(lv) for lv in levels) * BL <= 64
    assert lv_of[T - 1] == nlev - 1
    return levels, order, lv_of, idx_of, d1, d2


def _build(levels, lv_of, idx_of, d1, d2, order):
    nc = bacc.Bacc(None)

    NROW = len(order)
    MT = (NROW * BL + 127) // 128  # phase-1 M-tiles
    NPAD = MT * 128

    xs_d = nc.dram_tensor("xs", [2, 128, NPAD], bf16, kind="ExternalInput")
    wih_d = nc.dram_tensor("wih", [2, 128, G3], bf16, kind="ExternalInput")
    whh_d = nc.dram_tensor("whh", [KC, 128, G3], bf16, kind="ExternalInput")
    biasg_d = nc.dram_tensor("biasg", [128, G3], f32, kind="ExternalInput")
    bias2_d = nc.dram_tensor("bias2", [128, HALF], bf16, kind="ExternalInput")
    # ident cols 0:128 = I_128; 128:192 = [I64;0]; 192:256 = [0;I64]
    ident_d = nc.dram_tensor("ident", [128, 256], bf16, kind="ExternalInput")
    out_d = nc.dram_tensor("out", [BL, H], f32, kind="ExternalOutput")
    xi_d = nc.dram_tensor("xi_scratch", [NPAD, G3], bf16)

    NLEV = len(levels)
    if NLEV_CAP:
        NLEV = min(NLEV, NLEV_CAP)
        levels = levels[:NLEV]
    t_out = T - 1

    with tile.TileContext(nc) as tc:
        with (
            tc.tile_pool(name="rec", bufs=1) as rpool,
            tc.tile_pool(name="ps", bufs=7, space="PSUM") as pspool,
            tc.tile_pool(name="junk", bufs=1, space="PSUM") as junkpool,
        ):
            whh = rpool.tile([128, KC * G3], bf16)  # 48 KB/partition
            b2bc = rpool.tile([128, HALF], bf16)
            nc.sync.dma_start(b2bc[:], bias2_d[:])
            ident = rpool.tile([128, 256], bf16)
            nc.sync.dma_start(ident[:], ident_d[:])
            # hist^T: [128, slot(16) x chunk(8) x node-col(64)] bf16
            hist = rpool.tile([128, NLEVH * KC * WCOL], bf16)
            hv = hist[:].rearrange("p (s c j) -> p s c j", s=NLEVH, c=KC)

            # ---------- phase 1 (lazy): xi = x @ W_ih.T + bias ----------
            # emitted as (M-tile, bank) jobs interleaved into the level-loop
            # tail bubbles; they fill PE idle time and keep the clock warm
            wih = rpool.tile([128, 2 * G3], bf16)
            nc.sync.dma_start(
                wih[:].rearrange("p (k f) -> p k f", k=2),
                wih_d.rearrange("k p f -> p k f"),
            )
            xs = rpool.tile([128, 2 * NPAD], bf16)
            nc.sync.dma_start(
                xs[:].rearrange("p (k f) -> p k f", k=2),
                xs_d.rearrange("k p f -> p k f"),
            )
            biasg = rpool.tile([128, G3], f32)
            nc.sync.dma_start(biasg[:], biasg_d[:])
            # whh load on the SWDGE queue so the big transfer streams in
            # parallel with the sync-queue phase-1 flow (xi writes/loads)
            nc.gpsimd.dma_start(
                whh[:].rearrange("p (k f) -> p k f", k=KC),
                whh_d.rearrange("k p f -> p k f"),
            )
            p1sb_rot = [rpool.tile([128, 512], bf16, name=f"p1sb{i}") for i in range(3)]
            p1_state = [0]

            p1_pending = []

            def emit_p1_mm():
                j = p1_state[0]
                if j >= MT * 6 or len(p1_pending) >= 2:
                    return False
                p1_state[0] = j + 1
                m, nb = divmod(j, 6)
                ps = pspool.tile([128, 512], f32, tag="ps", name="p1ps")
                for k in range(2):
                    nc.tensor.matmul(
                        ps[:],
                        xs[:, k * NPAD + m * 128 : k * NPAD + (m + 1) * 128],
                        wih[:, k * G3 + nb * 512 : k * G3 + (nb + 1) * 512],
                        start=(k == 0),
                        stop=(k == 1),
                    )
                p1_pending.append((j, m, nb, ps))
                return True

            def flush_p1():
                while p1_pending:
                    j, m, nb, ps = p1_pending.pop(0)
                    sb = p1sb_rot[j % 3]
                    nc.vector.tensor_add(
                        sb[:], ps[:], biasg[:, nb * 512 : (nb + 1) * 512]
                    )
                    nc.sync.dma_start(
                        xi_d[m * 128 : (m + 1) * 128, nb * 512 : (nb + 1) * 512],
                        sb[:],
                    )

            def emit_p1_job():
                if emit_p1_mm():
                    flush_p1()
                    return True
                return False

            def p1_rows_done():
                return (p1_state[0] // 6) * 128

            # ---------- phase 2: recurrence over DAG levels ----------
            with (
                tc.tile_pool(name="rzp", bufs=2) as rzp,
                tc.tile_pool(name="npp", bufs=2) as npp,
                tc.tile_pool(name="hbm", bufs=2) as hbmp,
                tc.tile_pool(name="hnp", bufs=3) as hnp,
                tc.tile_pool(name="outp", bufs=1) as opool,
            ):
                hbl_rot = [
                    rpool.tile([128, KC * WCOL], bf16, name=f"hblr{i}")
                    for i in range(3)
                ]
                xi_rot = [
                    rpool.tile([128, 3 * HALF], bf16, name=f"xir{i}")
                    for i in range(3)
                ]

                def hist_pair(t, c0):
                    """[128, 2, BL] view of dep t's h^T chunks c0, c0+1."""
                    sl = int(lv_of[t]) % NLEVH
                    j = idx_of[t]
                    return hv[:, sl, c0 : c0 + 2, j * BL : (j + 1) * BL]

                def emit_blend(eng, hbl, nodes, half):
                    """Blend deps into hbl chunks {0,1,4,5} (half 0) or
                    {2,3,6,7} (half 1); two 2-chunk ops per node."""
                    hblv = hbl[:].rearrange("p (c j) -> p c j", c=KC)
                    for c0 in ((0, 4) if half == 0 else (2, 6)):
                        for i, t in enumerate(nodes):
                            dst = hblv[:, c0 : c0 + 2, i * BL : (i + 1) * BL]
                            a, b2 = int(d1[t]), int(d2[t])
                            if a < 0 and b2 < 0:
                                eng.memset(dst, 0.0)
                            elif a >= 0 and b2 >= 0:
                                eng.tensor_add(
                                    dst, hist_pair(a, c0), hist_pair(b2, c0)
                                )
                            else:
                                eng.tensor_copy(
                                    dst, hist_pair(a if a >= 0 else b2, c0)
                                )

                # row offset of each level's xi rows (level-sorted, tight)
                srow = [0]
                for lvw in levels:
                    srow.append(srow[-1] + len(lvw))

                junk = junkpool.tile([128, 512], f32, tag="junk")

                def warm(mv):
                    # tiny matmul whose moving operand is a tail output: it
                    # fires mid-bubble (right after that output lands) and
                    # keeps the PE activity monitor from re-throttling
                    nc.tensor.matmul(
                        junk[0:64, 0 : mv.shape[-1]], whh[:, 0:64], mv,
                        start=True, stop=True, skip_group_check=True,
                    )

                # pre-loop: zero all rotation buffers (garbage lanes must
                # hold written data for the full-64-wide matmul reads)
                for tbuf in hbl_rot:
                    nc.vector.memset(tbuf[:, :], 0.0)
                for tbuf in xi_rot:
                    nc.gpsimd.memset(tbuf[:, :], 0.0)
                hbl_cur = hbl_rot[0]
                emit_blend(nc.vector, hbl_cur, levels[0], 0)
                emit_blend(nc.gpsimd, hbl_cur, levels[0], 1)

                # 3 xi tiles up front (covers first ~5 levels); after the
                # memsets/blend-0 so they don't block level-0 on the DVE queue
                for _ in range(18):
                    emit_p1_job()

                # chunk stream order: h1-chunks first (their blend lands first)
                CORD = (0, 1, 4, 5, 2, 3, 6, 7)

                for lv, nodes in enumerate(levels):
                    w = len(nodes)
                    M = BL * w
                    sl = lv % NLEVH
                    hbl = hbl_cur

                    # xi rows for this level (SWDGE, off the HWDGE queues)
                    # [0:M] rows = gate cols 0:1536 (rA zA nA),
                    # [64:64+M]  = gate cols 1536:3072 (rB zB nB)
                    xi = xi_rot[lv % 3]
                    r0 = srow[lv] * BL
                    nc.sync.dma_start(xi[0:M, :], xi_d[r0 : r0 + M, 0 : 3 * HALF])
                    nc.sync.dma_start(
                        xi[64 : 64 + M, :], xi_d[r0 : r0 + M, 3 * HALF : G3]
                    )

                    ps_hb = pspool.tile([128, 512], f32, tag="ps", name="ps_hb")
                    ps_r = pspool.tile([128, 512], f32, tag="ps", name="ps_r")
                    ps_n = pspool.tile([128, 512], f32, tag="ps", name="ps_n")
                    ps_z = pspool.tile([128, 512], f32, tag="ps", name="ps_z")

                    def gate_stream(psb, gcol, lo=0, hi=512):
                        for ci, c in enumerate(CORD):
                            lhsT = hbl[:, c * WCOL : (c + 1) * WCOL]
                            st = ci == 0
                            sp = ci == KC - 1
                            nc.tensor.matmul(
                                psb[0:64, lo:hi],
                                lhsT,
                                whh[:, c * G3 + gcol * 512 + lo : c * G3 + gcol * 512 + hi],
                                start=st,
                                stop=sp,
                                skip_group_check=True,
                            )
                            nc.tensor.matmul(
                                psb[64:128, lo:hi],
                                lhsT,
                                whh[:, c * G3 + 1536 + gcol * 512 + lo : c * G3 + 1536 + gcol * 512 + hi],
                                start=st,
                                stop=sp,
                                skip_group_check=True,
                            )

                    # ---- r/n gates: h1 chunks of both banks first, so the
                    # gpsimd blend-h2 of this level gets ~1.7us of lead ----
                    def gate_stream_half(psb, gcol, half):
                        for ci, c in enumerate(CORD):
                            if (ci < 4) != (half == 0):
                                continue
                            lhsT = hbl[:, c * WCOL : (c + 1) * WCOL]
                            st = ci == 0
                            sp = ci == KC - 1
                            nc.tensor.matmul(
                                psb[0:64, :],
                                lhsT,
                                whh[:, c * G3 + gcol * 512 : c * G3 + (gcol + 1) * 512],
                                start=st, stop=sp, skip_group_check=True,
                            )
                            nc.tensor.matmul(
                                psb[64:128, :],
                                lhsT,
                                whh[:, c * G3 + 1536 + gcol * 512 : c * G3 + 1536 + (gcol + 1) * 512],
                                start=st, stop=sp, skip_group_check=True,
                            )

                    gate_stream_half(ps_r, 0, 0)
                    gate_stream_half(ps_n, 2, 0)
                    gate_stream_half(ps_r, 0, 1)
                    gate_stream_half(ps_n, 2, 1)

                    # ---- h_blend batch layout: PE transposes of hbl ----
                    # chunk c (A: 0-3 / B: 4-7) -> ps_hb rows [0:M]/[64:64+M]
                    for c in range(4):
                        nc.tensor.matmul(
                            ps_hb[0:64, c * 128 : (c + 1) * 128],
                            hbl[:, c * WCOL : (c + 1) * WCOL],
                            ident[:, 0:128],
                            start=True, stop=True, skip_group_check=True,
                        )
                        nc.tensor.matmul(
                            ps_hb[64:128, c * 128 : (c + 1) * 128],
                            hbl[:, (c + 4) * WCOL : (c + 5) * WCOL],
                            ident[:, 0:128],
                            start=True, stop=True, skip_group_check=True,
                        )

                    rz = rzp.tile([128, 2 * 512], bf16, tag="rz")
                    np_ = npp.tile([128, 512], bf16, tag="np")
                    hbmn = hbmp.tile([128, 512], bf16, tag="hbmn")
                    hnew = hnp.tile([128, 512], bf16, tag="hnew")

                    # r = sigmoid(ps_r + xi_r)  (overlaps z streams)
                    nc.vector.tensor_add(rz[:, 0:512], ps_r[:, :], xi[:, 0:512])
                    nc.scalar.activation(rz[:, 0:512], rz[:, 0:512], AF.Sigmoid)

                    # ---- z gate, xi_z folded via identity selectors ----
                    gate_stream(ps_z, 1)
                    nc.tensor.matmul(
                        ps_z[0:64, :],
                        ident[:, 128:192],
                        xi[:, 512:1024],
                        start=False, stop=False, skip_group_check=True,
                    )
                    nc.tensor.matmul(
                        ps_z[64:128, :],
                        ident[:, 192:256],
                        xi[:, 512:1024],
                        start=False, stop=False, skip_group_check=True,
                    )

                    # hb psum -> sbuf early (off the critical chain, on ACT)
                    nc.scalar.copy(hbmn[:, :], ps_hb[:, :])
                    # n = tanh(xi_n + r * (ps_n + b_hh_n))
                    nc.vector.tensor_add(np_[:, :], ps_n[:, :], b2bc[:, :])
                    nc.vector.tensor_mul(np_[:, :], np_[:, :], rz[:, 0:512])
                    nc.vector.tensor_add(np_[:, :], np_[:, :], xi[:, 1024:1536])
                    nc.scalar.activation(np_[:, 0:256], np_[:, 0:256], AF.Tanh)
                    nc.scalar.activation(np_[:, 256:512], np_[:, 256:512], AF.Tanh)
                    nc.scalar.activation(rz[:, 512:768], ps_z[:, 0:256], AF.Sigmoid)
                    nc.scalar.activation(rz[:, 768:1024], ps_z[:, 256:512], AF.Sigmoid)
                    warm(np_[:, 0:256])
                    if not emit_p1_mm():
                        nc.tensor.matmul(
                            junk[0:64, :], whh[:, 0:64], whh[:, 0:512],
                            start=True, stop=True, skip_group_check=True,
                        )
                    warm(rz[:, 512:768])
                    if not emit_p1_mm():
                        nc.tensor.matmul(
                            junk[0:64, :], whh[:, 0:64], whh[:, 0:512],
                            start=True, stop=True, skip_group_check=True,
                        )

                    # h_new = n + z*(hb-n), in halves; each half's transposes,
                    # hist copies and next-level blends follow immediately
                    for hh in range(2):
                        cs = slice(hh * 256, (hh + 1) * 256)
                        nc.vector.tensor_sub(hbmn[:, cs], hbmn[:, cs], np_[:, cs])
                        nc.vector.tensor_mul(
                            hbmn[:, cs], hbmn[:, cs],
                            rz[:, 512 + hh * 256 : 512 + (hh + 1) * 256],
                        )
                        nc.vector.tensor_add(hnew[:, cs], hbmn[:, cs], np_[:, cs])
                        warm(hnew[:, cs])

                        if lv + 1 < NLEV:
                            pt = pspool.tile([128, 256], f32, tag="ps", name="pt")
                            for k in range(2):
                                col = hh * 256 + k * 128
                                nc.tensor.matmul(
                                    pt[:, k * WCOL : (k + 1) * WCOL],
                                    hnew[:, col : col + 128],
                                    ident[:, 128:192],
                                    start=True, stop=True,
                                    skip_group_check=True,
                                )
                                nc.tensor.matmul(
                                    pt[:, (2 + k) * WCOL : (3 + k) * WCOL],
                                    hnew[:, col : col + 128],
                                    ident[:, 192:256],
                                    start=True, stop=True,
                                    skip_group_check=True,
                                )
                            # psum -> hist^T: chunks {hh*2, hh*2+1} and
                            # {hh*2+4, hh*2+5} -- one strided copy
                            base = sl * KC * WCOL
                            hd = hist[
                                :, base + hh * 128 : base + hh * 128 + 384
                            ].rearrange("p (g f) -> p g f", g=3)[:, 0:3:2, :]
                            nc.scalar.copy(
                                hd,
                                pt[:].rearrange("p (g f) -> p g f", g=2),
                            )
                            if hh == 0:
                                hbl_cur = hbl_rot[(lv + 1) % 3]

                    if lv + 1 < NLEV:
                        emit_blend(nc.vector, hbl_cur, levels[lv + 1], 0)
                        emit_blend(nc.gpsimd, hbl_cur, levels[lv + 1], 1)

                    flush_p1()
                    need = srow[min(lv + 5, NLEV)] * BL
                    emitted = 0
                    while p1_rows_done() < min(need + 128, NPAD) and emitted < 6:
                        if not emit_p1_job():
                            break
                        emitted += 1

                    if lv == NLEV - 1:
                        j = idx_of[t_out] if not NLEV_CAP else 0
                        outt = opool.tile([128, HALF], f32, tag="outt")
                        nc.vector.tensor_scalar_mul(outt[:, :], hnew[:, :], 2.0)
                        nc.sync.dma_start(
                            out_d[:, 0:HALF], outt[j * BL : (j + 1) * BL, :]
                        )
                        nc.sync.dma_start(
                            out_d[:, HALF:H], outt[64 + j * BL : 64 + (j + 1) * BL, :]
                        )

    nc.finalize()
    return nc


def kernel(**inputs):
    x = np.asarray(inputs["x"], np.float32)
    W_ih = np.asarray(inputs["W_ih"], np.float32)
    W_hh = np.asarray(inputs["W_hh"], np.float32)
    b_ih = np.asarray(inputs["b_ih"], np.float32)
    b_hh = np.asarray(inputs["b_hh"], np.float32)
    w1 = np.asarray(inputs["w1"], np.int32)
    w2 = np.asarray(inputs["w2"], np.int32)
    assert int(inputs["skip_size"]) == SKIP

    levels, order, lv_of, idx_of, d1, d2 = _plan(w1, w2)
    nc = _build(levels, lv_of, idx_of, d1, d2, order)

    NROW = len(order)
    MT = (NROW * BL + 127) // 128
    NPAD = MT * 128

    # gate-column permutation: [rA zA nA rB zB nB], A = cols 0:512 of a gate
    perm = np.concatenate(
        [
            np.arange(0, HALF),
            np.arange(H, H + HALF),
            np.arange(2 * H, 2 * H + HALF),
            np.arange(HALF, H),
            np.arange(H + HALF, 2 * H),
            np.arange(2 * H + HALF, 3 * H),
        ]
    )
    W_hh_p = W_hh[perm]
    W_ih_p = W_ih[perm]
    bias = (b_ih + b_hh).copy()
    bias[2 * H :] = b_ih[2 * H :]  # n-part: only b_ih (b_hh_n inside r*(.))
    bias = bias[perm]
    whh_t = np.ascontiguousarray(W_hh_p.T.reshape(KC, 128, G3)).astype(bfnp)
    wih_t = np.ascontiguousarray(W_ih_p.T.reshape(2, 128, G3)).astype(bfnp)
    biasg = np.broadcast_to(bias, (128, G3)).astype(np.float32).copy()
    # b_hh_n partition-packed: rows 0:64 = cols 0:512, rows 64:128 = 512:1024
    bias2g = np.zeros((128, HALF), dtype=bfnp)
    bias2g[0:64, :] = b_hh[2 * H : 2 * H + HALF].astype(bfnp)
    bias2g[64:128, :] = b_hh[2 * H + HALF : 3 * H].astype(bfnp)
    identh = np.zeros((128, 256), dtype=bfnp)
    identh[:, 0:128] = np.eye(128, dtype=bfnp)
    identh[0:64, 128:192] = np.eye(64, dtype=bfnp)
    identh[64:128, 192:256] = np.eye(64, dtype=bfnp)
    in_maps = []
    for c in range(NCORES):
        xc = x[c * BL : (c + 1) * BL]  # [8, T, X]
        xsrt = xc[:, order, :]  # pruned, level-sorted: [8, NROW, 256]
        xs = xsrt.transpose(2, 1, 0).reshape(2, 128, NROW * BL)
        xsp = np.zeros((2, 128, NPAD), np.float32)
        xsp[:, :, : NROW * BL] = xs
        in_maps.append(
            {
                "xs": xsp.astype(bfnp),
                "wih": wih_t,
                "whh": whh_t,
                "biasg": biasg,
                "bias2": bias2g,
                "ident": identh,
            }
        )
    res = None
    for attempt in range(3):
        try:
            res = run_bass_kernel_spmd(nc, in_maps, core_ids=list(range(NCORES)))
            break
        except Exception:
            if attempt == 2:
                raise
    if getattr(res, "exec_time_ns", None):
        print("HW exec time:", res.exec_time_ns, "ns")
    global LAST_RESULT
    LAST_RESULT = res
    out = np.concatenate([res.results[c]["out"] for c in range(NCORES)], axis=0)
    return np.asarray(out, np.float32)


LAST_RESULT = None
